# revision 29
# baseline (speedup 1.0000x reference)
# CondConv2d Trainium2 kernel (v3).
#
# Math (per sample n=(b,l)):
#   pooled[c]   = mean_{h,w} x[n,c,h,w]
#   allxet      = [p0,p0,p0,p1,p2,p3] temporal window (first frame dup'd twice)
#   calib[c,t]  = conv1d(allxet, tconv_w)[c,t] + tconv_b[c]
#   gate[t]     = conv1d(allxet, fc_w)[0,t] + fc_b
#   scale[n,c]  = calib[c,l] + 1
#   out[n,o]    = conv2d(x[n] * scale[n,:,None,None], weight) + bias[o]*(gate[l]+1)
# (the per-sample weight scale is folded into the input because conv is
#  linear in each input channel)
#
# Sharding: data-parallel over b: 8 cores x 2 batch entries (8 (b,l)
# samples per core). Weights replicated.
#
# Perf notes (from v1/v2 traces):
#  - Each engine owns ONE serial DMA queue; a second dma_start on the same
#    engine blocks its instruction stream until the first transfer
#    completes. So: no DMAs on vector/scalar (they compute), x on sync,
#    params on gpsimd (+ w[oc0] as the single tensor-queue DMA), stores
#    alternate sync/gpsimd.
#  - Per-queue DMA throughput scales with the per-partition line size, so
#    every tensor is staged partition-major with 4-9KB contiguous lines
#    (output is written partition-major and un-permuted on the host).
#  - fp32 matmuls lower to LOW/HIGH microinstruction pairs (~1.1us per
#    tiny matmul); the whole calib/gate path therefore runs in bf16 off a
#    bf16-cast mirror of the pooled values.
#  - conv is bf16 (FWL weight loads hide fully); kh AND kw edge clipping
#    skips the zero halo; fp32 warmup matmuls bridge the DMA window so
#    HAM never throttles mid-kernel.

import numpy as np
from ml_dtypes import bfloat16 as np_bf16


def _install_axon_ntff_shim():
    # This container's `antenv` stub lacks `axon_hooks`, which
    # bass_utils imports unconditionally when trace=True under axon.
    import os
    import sys
    import types

    try:
        import antenv.axon_hooks  # noqa: F401

        return
    except Exception:
        pass
    try:
        import antenv
    except Exception:
        return
    mod = types.ModuleType("antenv.axon_hooks")
    mod._hook = None

    def set_axon_ntff_profile_hook(h):
        mod._hook = h

    def get_axon_ntff_profile_hook():
        return mod._hook

    mod.set_axon_ntff_profile_hook = set_axon_ntff_profile_hook
    mod.get_axon_ntff_profile_hook = get_axon_ntff_profile_hook
    sys.modules["antenv.axon_hooks"] = mod
    antenv.axon_hooks = mod
    try:
        from trn_agent_boot.trn_boot import _ntff_profile_via_ctypes

        so = "/opt/axon/libaxon_pjrt.so"
        if os.path.exists(so):
            mod._hook = _ntff_profile_via_ctypes(so)
    except Exception:
        pass


_install_axon_ntff_shim()

import concourse.bass as bass
import concourse.tile as tile
from concourse import mybir
from concourse.bass_utils import run_bass_kernel_spmd

B, L, CIN, COUT, KS, H, W = 16, 4, 256, 256, 3, 32, 32
NCORES = 8
BS = B // NCORES      # batch entries per core
CC = CIN // 128       # ci chunks
OC = COUT // 128      # co chunks
FP32 = mybir.dt.float32
BF16 = mybir.dt.bfloat16
HH = 16               # psum bank = 512 fp32 = 16 rows of 32
N_WARM = 5            # fp32 warmup matmuls (~1.05us each at cold clock)

_last_results = None  # test harness reads exec_time_ns from here


def _split_excess_waits(nc):
    # walrus in this toolchain encodes exactly one sem wait per engine
    # instruction (TPB_EVENTS has a single wait slot) and optimize_sems
    # is disabled, so Tile can emit instructions with >1 wait that fail
    # codegen ("Too many sync wait commands").  Split the excess waits
    # into standalone EventSemaphore instructions on the same engine
    # stream immediately before the instruction; in-order issue makes
    # this equivalent.
    n = 0
    f = nc.m.functions[0]
    for bb in f.blocks:
        insts = list(bb.instructions)
        out = []
        changed = False
        for inst in insts:
            si = inst.sync_info
            if si is not None:
                waits = list(si.on_wait)
                if len(waits) > 1:
                    for w in waits[:-1]:
                        n += 1
                        es = mybir.InstEventSemaphore(name=f"ES-SPLIT-{n}")
                        es.engine = inst.engine
                        es.sync_info = mybir.SyncInfo(on_wait=[w], on_update=[])
                        out.append(es)
                    si.on_wait = [waits[-1]]
                    inst.sync_info = si
                    changed = True
            out.append(inst)
        if changed:
            bb.instructions = out
    return n


def build_nc():
    nc = bass.Bass()
    # b0 frames l-major (per-l DMAs for fast start), b1 partition-major
    # (one big 8KB-line DMA; only needed ~20us in)
    x0_d = nc.dram_tensor("x0", [L, 128, CC, H, W], BF16, kind="ExternalInput")
    x1_d = nc.dram_tensor("x1", [128, L, CC, H, W], BF16, kind="ExternalInput")
    w_d = nc.dram_tensor("w", [128, OC, CC, 9, 128], BF16, kind="ExternalInput")
    # tconv weights with the fc (gate) weights folded in as out-channel CIN
    tcw_d = nc.dram_tensor("tcwfcw", [128, CC, 3, CIN + 1], BF16,
                           kind="ExternalInput")
    # [tb1 (CC) | bias2 (OC) | fcb1 (1)]
    sm_d = nc.dram_tensor("smalls", [128, CC + OC + 1], FP32,
                          kind="ExternalInput")
    # partition-major output, un-permuted on the host
    out_d = nc.dram_tensor("out", [BS, L, 128, OC, H, W], FP32,
                           kind="ExternalOutput")

    with tile.TileContext(nc) as tc:
        with (
            tc.tile_pool(name="singles", bufs=1) as singles,
            tc.tile_pool(name="outp", bufs=3) as outp,
            tc.tile_pool(name="pp_conv", bufs=6, space="PSUM") as pp_conv,
            tc.tile_pool(name="pp_small", bufs=2, space="PSUM") as pp_small,
        ):
            # ---- persistent tiles ----
            w_sb = singles.tile([128, OC, CC, 9, 128], BF16, tag="w")
            tcw_sb = singles.tile([128, CC, 3, CIN + 1], BF16, tag="tcw")
            sm_sb = singles.tile([128, CC + OC + 1], FP32, tag="smalls")
            ones_sb = singles.tile([1, 128], BF16, tag="ones")
            warm_sb = singles.tile([128, 512], FP32, tag="warm")

            allxet = singles.tile([128, CC, BS, L + 2], FP32, tag="allxet")
            allxet_bf = singles.tile([128, CC, BS, L + 2], BF16, tag="allxet_bf")
            s_sb = singles.tile([128, CC, BS, L], FP32, tag="s")
            g_sb = singles.tile([1, BS, L], BF16, tag="g")
            fb_sb = singles.tile([128, BS, L, OC], FP32, tag="fb")

            xr0 = {}
            for l in range(L):
                xr = singles.tile([128, CC, H, W], BF16, tag=f"xr0_{l}")
                xr0[l] = xr
            xr1 = singles.tile([128, L, CC, H, W], BF16, tag="xr1")
            pscr = singles.tile([128, H, W], BF16, tag="pool_scratch")
            x_t = {}
            for b in range(BS):
                for l in range(L):
                    for ci in range(CC):
                        xt = singles.tile([128, H, W], BF16, tag=f"xt{b}_{l}_{ci}")
                        x_t[(b, l, ci)] = xt

            def xr_ap(b, l, ci):
                if b == 0:
                    return xr0[l][:, ci]
                return xr1[:, l, ci]

            tb1_ap = lambda oc: sm_sb[:, oc:oc + 1]
            bias_ap = lambda oc: sm_sb[:, CC + oc:CC + oc + 1]
            fcb1_ap = sm_sb[0:1, CC + OC:CC + OC + 1]

            # ---- t=0: DMAs spread across queues, tiny vector setup ----
            nc.vector.memset(warm_sb[:], 0.0)
            nc.vector.memset(ones_sb[:], 1.0)

            # a second dma_start on a queue blocks that engine until the
            # first transfer completes, so: scalar gets exactly one DMA
            # (w[oc0], gating the first conv), gpsimd's tcw leads so
            # calibA isn't gated, sync streams the frames in use-order
            nc.scalar.dma_start(out=w_sb[:, 0], in_=w_d[:, 0])

            nc.gpsimd.dma_start(out=tcw_sb[:], in_=tcw_d[:])
            nc.gpsimd.dma_start(out=sm_sb[:], in_=sm_d[:])
            nc.gpsimd.dma_start(out=w_sb[:, 1], in_=w_d[:, 1])

            nc.sync.dma_start(out=xr0[0][:], in_=x0_d[0])
            nc.sync.dma_start(out=xr0[1][:], in_=x0_d[1])
            nc.sync.dma_start(out=xr0[2][:], in_=x0_d[2])
            nc.sync.dma_start(out=xr0[3][:], in_=x0_d[3])
            nc.sync.dma_start(out=xr1[:], in_=x1_d[:])

            # ---- tensor: warmup matmuls (HAM stays un-throttled) ----
            for _ in range(N_WARM):
                wps = pp_conv.tile([128, HH, W], FP32, tag="conv")
                nc.tensor.matmul(
                    wps[:], lhsT=warm_sb[:, 0:128], rhs=warm_sb[:],
                    start=True, stop=True,
                )

            def pool(b, l, ci, eng="v"):
                if eng == "v":
                    nc.vector.reduce_sum(
                        out=allxet[:, ci, b, 2 + l:3 + l],
                        in_=xr_ap(b, l, ci),
                        axis=mybir.AxisListType.XY,
                    )
                else:
                    # scalar-engine pool: ACT copy with free-dim accumulate
                    nc.scalar.activation(
                        pscr[:], xr_ap(b, l, ci),
                        mybir.ActivationFunctionType.Copy,
                        accum_out=allxet[:, ci, b, 2 + l:3 + l],
                    )

            def dup_first(b, ci):
                nc.vector.tensor_copy(allxet[:, ci, b, 0:1], allxet[:, ci, b, 2:3])
                nc.vector.tensor_copy(allxet[:, ci, b, 1:2], allxet[:, ci, b, 2:3])

            # b0 l0 pools on vector; cast frame-0's window columns
            pool(0, 0, 0)
            pool(0, 0, 1)
            dup_first(0, 0)
            dup_first(0, 1)
            nc.vector.tensor_copy(
                allxet_bf[:, :, 0:1, 0:3], allxet[:, :, 0:1, 0:3]
            )

            def calib(bA, bN, l0, nl):
                # calib for batch entries bA..bA+bN-1, frames l0..l0+nl-1
                for oc in range(CC):
                    pc = pp_small.tile([128, BS, L], FP32, tag="small")
                    mms = [(ci, k) for ci in range(CC) for k in range(3)]
                    for i, (ci, k) in enumerate(mms):
                        nc.tensor.matmul(
                            pc[:, 0:bN, 0:nl],
                            lhsT=tcw_sb[:, ci, k, oc * 128:(oc + 1) * 128],
                            rhs=allxet_bf[:, ci, bA:bA + bN, k + l0:k + l0 + nl],
                            start=(i == 0),
                            stop=(i == len(mms) - 1),
                        )
                    nc.vector.tensor_scalar_add(
                        s_sb[:, oc, bA:bA + bN, l0:l0 + nl],
                        pc[:, 0:bN, 0:nl], tb1_ap(oc),
                    )

            def gate(bA, bN, l0, nl, fb_list):
                # gate conv1d for entries bA..bA+bN-1, frames l0..l0+nl-1;
                # fb (bias * (gate+1)) written only for fb_list pairs
                pg = pp_small.tile([128, BS, L], FP32, tag="small")
                mms = [(ci, k) for ci in range(CC) for k in range(3)]
                for i, (ci, k) in enumerate(mms):
                    nc.tensor.matmul(
                        pg[0:1, 0:bN, 0:nl],
                        lhsT=tcw_sb[:, ci, k, CIN:CIN + 1],
                        rhs=allxet_bf[:, ci, bA:bA + bN, k + l0:k + l0 + nl],
                        start=(i == 0),
                        stop=(i == len(mms) - 1),
                    )
                nc.vector.tensor_scalar_add(
                    g_sb[0:1, bA:bA + bN, l0:l0 + nl], pg[0:1, 0:bN, 0:nl],
                    fcb1_ap,
                )
                gb = pp_small.tile([128, BS, L], FP32, tag="small")
                nc.tensor.matmul(
                    gb[:, 0:bN, 0:nl], lhsT=ones_sb[0:1, :],
                    rhs=g_sb[0:1, bA:bA + bN, l0:l0 + nl],
                    start=True, stop=True,
                )
                for b, l in fb_list:
                    for oc in range(OC):
                        nc.vector.tensor_mul(
                            fb_sb[:, b, l, oc:oc + 1],
                            gb[:, b - bA, l - l0:l - l0 + 1],
                            bias_ap(oc),
                        )

            def scale_x(b, l, ci, eng="s"):
                # per-(sample, ci-chunk) channel scale folded into x; the
                # output cast produces the bf16 matmul operand
                if eng == "v":
                    nc.vector.tensor_scalar_mul(
                        x_t[(b, l, ci)][:], xr_ap(b, l, ci),
                        s_sb[:, ci, b, l:l + 1],
                    )
                else:
                    nc.scalar.mul(
                        x_t[(b, l, ci)][:], xr_ap(b, l, ci),
                        s_sb[:, ci, b, l:l + 1],
                    )

            # frame-0 calib/gate/scale: conv (b0,l0) starts on one frame
            calib(0, 1, 0, 1)
            gate(0, 1, 0, 1, [(0, 0)])
            scale_x(0, 0, 0, "s")
            scale_x(0, 0, 1, "s")

            # frame 1 when it lands
            pool(0, 1, 0)
            pool(0, 1, 1)
            nc.vector.tensor_copy(
                allxet_bf[:, :, 0:1, 3:4], allxet[:, :, 0:1, 3:4]
            )
            calib(0, 1, 1, 1)
            gate(0, 1, 1, 1, [(0, 1)])
            scale_x(0, 1, 0, "s")
            scale_x(0, 1, 1, "s")

            # b0 l2/l3 pools (execute when those frames land)
            for l in (2, 3):
                pool(0, l, 0)
                pool(0, l, 1)

            # ---- conv groups ----
            sample_idx = [0]

            def conv_sample(b, l):
                osb = outp.tile([128, OC, H, W], FP32, tag="osb")
                for oc in range(OC):
                    for half in range(2):
                        h0 = half * HH
                        ps = pp_conv.tile([128, HH, W], FP32, tag="conv")
                        group = []
                        for ci in range(CC):
                            for kh in range(3):
                                dh = kh - 1
                                hA = max(h0, -dh)
                                hB = min(h0 + HH, H - dh)
                                for kw in range(3):
                                    dw = kw - 1
                                    cA = max(0, -dw)
                                    cB = min(W, W - dw)
                                    group.append((ci, dh, dw, hA, hB, cA, cB))
                        for i, (ci, dh, dw, hA, hB, cA, cB) in enumerate(group):
                            nc.tensor.matmul(
                                ps[:, hA - h0:hB - h0, cA:cB],
                                lhsT=w_sb[:, oc, ci, (dh + 1) * 3 + (dw + 1), :],
                                rhs=x_t[(b, l, ci)][
                                    :, hA + dh:hB + dh, cA + dw:cB + dw
                                ],
                                start=(i == 0),
                                stop=(i == len(group) - 1),
                            )
                        fb_ap = fb_sb[:, b, l, oc:oc + 1]
                        dst = osb[:, oc, h0:h0 + HH, :]
                        last = sample_idx[0] == BS * L - 1
                        # last sample: final half-add lands on vector (its
                        # queue is free) so the tail isn't serialized
                        on_v = (half == 0) if not last else ((oc + half) != 1)
                        if on_v:
                            nc.vector.tensor_scalar_add(dst, ps[:], fb_ap)
                        else:
                            nc.scalar.add(dst, ps[:], fb_ap)
                        if last:
                            # tail: store each half as soon as it's added;
                            # the final one rides sync so gpsimd can drain
                            st = nc.sync if (oc + half) % 2 == 0 else nc.gpsimd
                            st.dma_start(
                                out=out_d[b, l, :, oc, h0:h0 + HH, :],
                                in_=osb[:, oc, h0:h0 + HH, :],
                            )
                    if sample_idx[0] < BS * L - 1:
                        st_eng = (nc.gpsimd if (sample_idx[0] + oc) % 2 == 0
                                  else nc.sync)
                        st_eng.dma_start(
                            out=out_d[b, l, :, oc], in_=osb[:, oc]
                        )
                sample_idx[0] += 1

            # b1 pools split vector/scalar (frames land during b0l0's
            # conv); the full bf16 window cast follows
            for l in range(L):
                pool(1, l, 0, "v")
                pool(1, l, 1, "s")
                if l == 0:
                    dup_first(1, 0)
                    dup_first(1, 1)
            nc.vector.tensor_copy(allxet_bf[:], allxet[:])

            conv_sample(0, 0)

            # joint calib/gate over both entries, all frames; b0 re-writes
            # are identical values whose consumers already ran
            calib(0, BS, 0, L)
            gate(0, BS, 0, L,
                 [(0, 2), (0, 3)] + [(1, l) for l in range(L)])
            for b, l in [(0, 2), (0, 3), (1, 0), (1, 1), (1, 2), (1, 3)]:
                scale_x(b, l, 0, "v")
                scale_x(b, l, 1, "s")

            conv_sample(0, 1)
            conv_sample(0, 2)
            conv_sample(0, 3)
            for l in range(L):
                conv_sample(1, l)

    _split_excess_waits(nc)
    return nc


def kernel(x, weight, bias, tconv_w, tconv_b, fc_w, fc_b):
    global _last_results
    x = np.asarray(x, dtype=np.float32)
    weight = np.asarray(weight, dtype=np.float32)
    bias = np.asarray(bias, dtype=np.float32)
    tconv_w = np.asarray(tconv_w, dtype=np.float32)
    tconv_b = np.asarray(tconv_b, dtype=np.float32)
    fc_w = np.asarray(fc_w, dtype=np.float32)
    fc_b = np.asarray(fc_b, dtype=np.float32)

    HW = H * W
    # host-side packing (shared across cores); 1/(H*W) pooling norm and
    # the +1 biases folded here
    x_bf = x.astype(np_bf16).reshape(B, L, CC, 128, HW)
    w_host = np.ascontiguousarray(
        weight.transpose(1, 2, 3, 0)
        .reshape(CC, 128, 9, OC, 128)
        .transpose(1, 3, 0, 2, 4)
        .astype(np_bf16)
    )
    inv = np.float32(1.0 / HW)
    tcw = (tconv_w * inv).transpose(1, 2, 0)          # (CIN_in, 3, CIN_out)
    fcw = (fc_w[0] * inv)[:, :, None]                 # (CIN_in, 3, 1)
    tcw_host = np.ascontiguousarray(
        np.concatenate([tcw, fcw], axis=2)
        .reshape(CC, 128, 3, CIN + 1)
        .transpose(1, 0, 2, 3)
        .astype(np_bf16)
    )
    sm_host = np.ascontiguousarray(np.concatenate([
        tconv_b.reshape(CC, 128).T + np.float32(1.0),
        bias.reshape(OC, 128).T,
        np.full((128, 1), fc_b[0] + 1.0, dtype=np.float32),
    ], axis=1))

    nc = build_nc()
    in_maps = []
    for core in range(NCORES):
        xc = x_bf[core * BS:(core + 1) * BS]          # (BS, L, CC, 128, HW)
        in_maps.append({
            "x0": np.ascontiguousarray(xc[0].transpose(0, 2, 1, 3)),
            "x1": np.ascontiguousarray(xc[1].transpose(2, 0, 1, 3)),
            "w": w_host,
            "tcwfcw": tcw_host,
            "smalls": sm_host,
        })
    res = run_bass_kernel_spmd(nc, in_maps, core_ids=list(range(NCORES)))
    _last_results = res
    # out_d is [BS, L, 128, OC, H, W] partition-major -> un-permute
    outs = []
    for r in res.results:
        o = r["out"].reshape(BS, L, 128, OC, HW).transpose(0, 1, 3, 2, 4)
        outs.append(np.ascontiguousarray(o).reshape(BS * L, COUT, H, W))
    return np.concatenate(outs, axis=0)


# revision 35
# speedup vs baseline: 1.0205x; 1.0205x over previous
# CondConv2d Trainium2 kernel (v3).
#
# Math (per sample n=(b,l)):
#   pooled[c]   = mean_{h,w} x[n,c,h,w]
#   allxet      = [p0,p0,p0,p1,p2,p3] temporal window (first frame dup'd twice)
#   calib[c,t]  = conv1d(allxet, tconv_w)[c,t] + tconv_b[c]
#   gate[t]     = conv1d(allxet, fc_w)[0,t] + fc_b
#   scale[n,c]  = calib[c,l] + 1
#   out[n,o]    = conv2d(x[n] * scale[n,:,None,None], weight) + bias[o]*(gate[l]+1)
# (the per-sample weight scale is folded into the input because conv is
#  linear in each input channel)
#
# Sharding: data-parallel over b: 8 cores x 2 batch entries (8 (b,l)
# samples per core). Weights replicated.
#
# Perf notes (from v1/v2 traces):
#  - Each engine owns ONE serial DMA queue; a second dma_start on the same
#    engine blocks its instruction stream until the first transfer
#    completes. So: no DMAs on vector/scalar (they compute), x on sync,
#    params on gpsimd (+ w[oc0] as the single tensor-queue DMA), stores
#    alternate sync/gpsimd.
#  - Per-queue DMA throughput scales with the per-partition line size, so
#    every tensor is staged partition-major with 4-9KB contiguous lines
#    (output is written partition-major and un-permuted on the host).
#  - fp32 matmuls lower to LOW/HIGH microinstruction pairs (~1.1us per
#    tiny matmul); the whole calib/gate path therefore runs in bf16 off a
#    bf16-cast mirror of the pooled values.
#  - conv is bf16 (FWL weight loads hide fully); kh AND kw edge clipping
#    skips the zero halo; fp32 warmup matmuls bridge the DMA window so
#    HAM never throttles mid-kernel.

import numpy as np
from ml_dtypes import bfloat16 as np_bf16


def _install_axon_ntff_shim():
    # This container's `antenv` stub lacks `axon_hooks`, which
    # bass_utils imports unconditionally when trace=True under axon.
    import os
    import sys
    import types

    try:
        import antenv.axon_hooks  # noqa: F401

        return
    except Exception:
        pass
    try:
        import antenv
    except Exception:
        return
    mod = types.ModuleType("antenv.axon_hooks")
    mod._hook = None

    def set_axon_ntff_profile_hook(h):
        mod._hook = h

    def get_axon_ntff_profile_hook():
        return mod._hook

    mod.set_axon_ntff_profile_hook = set_axon_ntff_profile_hook
    mod.get_axon_ntff_profile_hook = get_axon_ntff_profile_hook
    sys.modules["antenv.axon_hooks"] = mod
    antenv.axon_hooks = mod
    try:
        from trn_agent_boot.trn_boot import _ntff_profile_via_ctypes

        so = "/opt/axon/libaxon_pjrt.so"
        if os.path.exists(so):
            mod._hook = _ntff_profile_via_ctypes(so)
    except Exception:
        pass


_install_axon_ntff_shim()

import concourse.bass as bass
import concourse.tile as tile
from concourse import mybir
from concourse.bass_utils import run_bass_kernel_spmd

B, L, CIN, COUT, KS, H, W = 16, 4, 256, 256, 3, 32, 32
NCORES = 8
BS = B // NCORES      # batch entries per core
CC = CIN // 128       # ci chunks
OC = COUT // 128      # co chunks
FP32 = mybir.dt.float32
BF16 = mybir.dt.bfloat16
HH = 16               # psum bank = 512 fp32 = 16 rows of 32
N_WARM = 5            # fp32 warmup matmuls (~1.05us each at cold clock)

_last_results = None  # test harness reads exec_time_ns from here


def _split_excess_waits(nc):
    # walrus in this toolchain encodes exactly one sem wait per engine
    # instruction (TPB_EVENTS has a single wait slot) and optimize_sems
    # is disabled, so Tile can emit instructions with >1 wait that fail
    # codegen ("Too many sync wait commands").  Split the excess waits
    # into standalone EventSemaphore instructions on the same engine
    # stream immediately before the instruction; in-order issue makes
    # this equivalent.
    n = 0
    f = nc.m.functions[0]
    for bb in f.blocks:
        insts = list(bb.instructions)
        out = []
        changed = False
        for inst in insts:
            si = inst.sync_info
            if si is not None:
                waits = list(si.on_wait)
                if len(waits) > 1:
                    for w in waits[:-1]:
                        n += 1
                        es = mybir.InstEventSemaphore(name=f"ES-SPLIT-{n}")
                        es.engine = inst.engine
                        es.sync_info = mybir.SyncInfo(on_wait=[w], on_update=[])
                        out.append(es)
                    si.on_wait = [waits[-1]]
                    inst.sync_info = si
                    changed = True
            out.append(inst)
        if changed:
            bb.instructions = out
    return n


def build_nc():
    nc = bass.Bass()
    # b0 frames l-major (per-l DMAs for fast start), b1 partition-major
    # (one big 8KB-line DMA; only needed ~20us in)
    x0_d = nc.dram_tensor("x0", [L, 128, CC, H, W], BF16, kind="ExternalInput")
    x1_d = nc.dram_tensor("x1", [128, L, CC, H, W], BF16, kind="ExternalInput")
    w_d = nc.dram_tensor("w", [128, OC, CC, 9, 128], BF16, kind="ExternalInput")
    # tconv weights with the fc (gate) weights folded in as out-channel CIN
    tcw_d = nc.dram_tensor("tcwfcw", [128, CC, 3, CIN + 1], BF16,
                           kind="ExternalInput")
    # [tb1 (CC) | bias2 (OC) | fcb1 (1)]
    sm_d = nc.dram_tensor("smalls", [128, CC + OC + 1], FP32,
                          kind="ExternalInput")
    # partition-major output, un-permuted on the host
    out_d = nc.dram_tensor("out", [BS, L, 128, OC, H, W], FP32,
                           kind="ExternalOutput")

    with tile.TileContext(nc) as tc:
        with (
            tc.tile_pool(name="singles", bufs=1) as singles,
            tc.tile_pool(name="outp", bufs=3) as outp,
            tc.tile_pool(name="pp_conv", bufs=6, space="PSUM") as pp_conv,
            tc.tile_pool(name="pp_small", bufs=2, space="PSUM") as pp_small,
        ):
            # ---- persistent tiles ----
            w_sb = singles.tile([128, OC, CC, 9, 128], BF16, tag="w")
            tcw_sb = singles.tile([128, CC, 3, CIN + 1], BF16, tag="tcw")
            sm_sb = singles.tile([128, CC + OC + 1], FP32, tag="smalls")
            ones_sb = singles.tile([1, 128], BF16, tag="ones")
            warm_sb = singles.tile([128, 512], FP32, tag="warm")

            allxet = singles.tile([128, CC, BS, L + 2], FP32, tag="allxet")
            allxet_bf = singles.tile([128, CC, BS, L + 2], BF16, tag="allxet_bf")
            s_sb = singles.tile([128, CC, BS, L], FP32, tag="s")
            g_sb = singles.tile([1, BS, L], BF16, tag="g")
            fb_sb = singles.tile([128, BS, L, OC], FP32, tag="fb")

            xr0 = {}
            for l in range(L):
                xr = singles.tile([128, CC, H, W], BF16, tag=f"xr0_{l}")
                xr0[l] = xr
            xr1 = singles.tile([128, L, CC, H, W], BF16, tag="xr1")
            pscr = singles.tile([128, H, W], BF16, tag="pool_scratch")
            x_t = {}
            for b in range(BS):
                for l in range(L):
                    for ci in range(CC):
                        xt = singles.tile([128, H, W], BF16, tag=f"xt{b}_{l}_{ci}")
                        x_t[(b, l, ci)] = xt

            def xr_ap(b, l, ci):
                if b == 0:
                    return xr0[l][:, ci]
                return xr1[:, l, ci]

            tb1_ap = lambda oc: sm_sb[:, oc:oc + 1]
            bias_ap = lambda oc: sm_sb[:, CC + oc:CC + oc + 1]
            fcb1_ap = sm_sb[0:1, CC + OC:CC + OC + 1]

            # ---- t=0: DMAs spread across queues, tiny vector setup ----
            nc.vector.memset(warm_sb[:], 0.0)
            nc.vector.memset(ones_sb[:], 1.0)

            # Queue speeds: sync/scalar ride HW DMA rings (~95GB/s per
            # transfer); gpsimd's is software-driven (~45GB/s) — it gets
            # only the tiny params tensor and the late-needed w[oc1].
            # A second dma_start on a queue blocks that engine until the
            # first transfer completes, so transfers are in need-order.
            nc.scalar.dma_start(out=tcw_sb[:], in_=tcw_d[:])
            nc.scalar.dma_start(out=w_sb[:, 0, 0], in_=w_d[:, 0, 0])
            nc.scalar.dma_start(out=w_sb[:, 0, 1], in_=w_d[:, 0, 1])

            nc.gpsimd.dma_start(out=sm_sb[:], in_=sm_d[:])
            nc.gpsimd.dma_start(out=w_sb[:, 1], in_=w_d[:, 1])

            nc.sync.dma_start(out=xr0[0][:], in_=x0_d[0])
            nc.sync.dma_start(out=xr0[1][:], in_=x0_d[1])
            nc.sync.dma_start(out=xr1[:], in_=x1_d[:])
            nc.sync.dma_start(out=xr0[2][:], in_=x0_d[2])
            nc.sync.dma_start(out=xr0[3][:], in_=x0_d[3])

            # ---- tensor: warmup matmuls (HAM stays un-throttled) ----
            for _ in range(N_WARM):
                wps = pp_conv.tile([128, HH, W], FP32, tag="conv")
                nc.tensor.matmul(
                    wps[:], lhsT=warm_sb[:, 0:128], rhs=warm_sb[:],
                    start=True, stop=True,
                )

            def pool(b, l, ci, eng="v"):
                if eng == "v":
                    nc.vector.reduce_sum(
                        out=allxet[:, ci, b, 2 + l:3 + l],
                        in_=xr_ap(b, l, ci),
                        axis=mybir.AxisListType.XY,
                    )
                else:
                    # scalar-engine pool: ACT copy with free-dim accumulate
                    nc.scalar.activation(
                        pscr[:], xr_ap(b, l, ci),
                        mybir.ActivationFunctionType.Copy,
                        accum_out=allxet[:, ci, b, 2 + l:3 + l],
                    )

            def dup_first(b, ci):
                nc.vector.tensor_copy(allxet[:, ci, b, 0:1], allxet[:, ci, b, 2:3])
                nc.vector.tensor_copy(allxet[:, ci, b, 1:2], allxet[:, ci, b, 2:3])

            # b0 l0 pools on vector; cast frame-0's window columns
            pool(0, 0, 0)
            pool(0, 0, 1)
            dup_first(0, 0)
            dup_first(0, 1)
            nc.vector.tensor_copy(
                allxet_bf[:, :, 0:1, 0:3], allxet[:, :, 0:1, 0:3]
            )

            def calib(bA, bN, l0, nl):
                # calib for batch entries bA..bA+bN-1, frames l0..l0+nl-1
                for oc in range(CC):
                    pc = pp_small.tile([128, BS, L], FP32, tag="small")
                    mms = [(ci, k) for ci in range(CC) for k in range(3)]
                    for i, (ci, k) in enumerate(mms):
                        nc.tensor.matmul(
                            pc[:, 0:bN, 0:nl],
                            lhsT=tcw_sb[:, ci, k, oc * 128:(oc + 1) * 128],
                            rhs=allxet_bf[:, ci, bA:bA + bN, k + l0:k + l0 + nl],
                            start=(i == 0),
                            stop=(i == len(mms) - 1),
                        )
                    nc.vector.tensor_scalar_add(
                        s_sb[:, oc, bA:bA + bN, l0:l0 + nl],
                        pc[:, 0:bN, 0:nl], tb1_ap(oc),
                    )

            def gate(bA, bN, l0, nl, fb_list):
                # gate conv1d for entries bA..bA+bN-1, frames l0..l0+nl-1;
                # fb (bias * (gate+1)) written only for fb_list pairs
                pg = pp_small.tile([128, BS, L], FP32, tag="small")
                mms = [(ci, k) for ci in range(CC) for k in range(3)]
                for i, (ci, k) in enumerate(mms):
                    nc.tensor.matmul(
                        pg[0:1, 0:bN, 0:nl],
                        lhsT=tcw_sb[:, ci, k, CIN:CIN + 1],
                        rhs=allxet_bf[:, ci, bA:bA + bN, k + l0:k + l0 + nl],
                        start=(i == 0),
                        stop=(i == len(mms) - 1),
                    )
                nc.vector.tensor_scalar_add(
                    g_sb[0:1, bA:bA + bN, l0:l0 + nl], pg[0:1, 0:bN, 0:nl],
                    fcb1_ap,
                )
                gb = pp_small.tile([128, BS, L], FP32, tag="small")
                nc.tensor.matmul(
                    gb[:, 0:bN, 0:nl], lhsT=ones_sb[0:1, :],
                    rhs=g_sb[0:1, bA:bA + bN, l0:l0 + nl],
                    start=True, stop=True,
                )
                for b, l in fb_list:
                    for oc in range(OC):
                        nc.vector.tensor_mul(
                            fb_sb[:, b, l, oc:oc + 1],
                            gb[:, b - bA, l - l0:l - l0 + 1],
                            bias_ap(oc),
                        )

            def scale_x(b, l, ci, eng="s"):
                # per-(sample, ci-chunk) channel scale folded into x; the
                # output cast produces the bf16 matmul operand
                if eng == "v":
                    nc.vector.tensor_scalar_mul(
                        x_t[(b, l, ci)][:], xr_ap(b, l, ci),
                        s_sb[:, ci, b, l:l + 1],
                    )
                else:
                    nc.scalar.mul(
                        x_t[(b, l, ci)][:], xr_ap(b, l, ci),
                        s_sb[:, ci, b, l:l + 1],
                    )

            # frame-0 calib/gate/scale: conv (b0,l0) starts on one frame;
            # scales on vector (scalar is DMA-issue-blocked early on)
            calib(0, 1, 0, 1)
            gate(0, 1, 0, 1, [(0, 0)])
            scale_x(0, 0, 0, "v")
            scale_x(0, 0, 1, "v")

            # frame 1 when it lands
            pool(0, 1, 0)
            pool(0, 1, 1)
            nc.vector.tensor_copy(
                allxet_bf[:, :, 0:1, 3:4], allxet[:, :, 0:1, 3:4]
            )
            calib(0, 1, 1, 1)
            gate(0, 1, 1, 1, [(0, 1)])
            scale_x(0, 1, 0, "s")
            scale_x(0, 1, 1, "s")

            # b1 pools (x1 lands third on sync); cast its window columns
            for l in range(L):
                pool(1, l, 0, "v")
                pool(1, l, 1, "s")
                if l == 0:
                    dup_first(1, 0)
                    dup_first(1, 1)
            nc.vector.tensor_copy(allxet_bf[:, :, 1:2, :], allxet[:, :, 1:2, :])

            # b0 l2/l3 pools + remaining window cast (frames land last)
            for l in (2, 3):
                pool(0, l, 0, "v")
                pool(0, l, 1, "s")
            nc.vector.tensor_copy(
                allxet_bf[:, :, 0:1, 4:6], allxet[:, :, 0:1, 4:6]
            )

            # ---- conv groups ----
            sample_idx = [0]

            def conv_sample(b, l, hooks=None):
                osb = outp.tile([128, OC, H, W], FP32, tag="osb")
                gidx = 0
                for oc in range(OC):
                    for half in range(2):
                        h0 = half * HH
                        ps = pp_conv.tile([128, HH, W], FP32, tag="conv")
                        group = []
                        for ci in range(CC):
                            for kh in range(3):
                                dh = kh - 1
                                hA = max(h0, -dh)
                                hB = min(h0 + HH, H - dh)
                                for kw in range(3):
                                    dw = kw - 1
                                    cA = max(0, -dw)
                                    cB = min(W, W - dw)
                                    group.append((ci, dh, dw, hA, hB, cA, cB))
                        for i, (ci, dh, dw, hA, hB, cA, cB) in enumerate(group):
                            nc.tensor.matmul(
                                ps[:, hA - h0:hB - h0, cA:cB],
                                lhsT=w_sb[:, oc, ci, (dh + 1) * 3 + (dw + 1), :],
                                rhs=x_t[(b, l, ci)][
                                    :, hA + dh:hB + dh, cA + dw:cB + dw
                                ],
                                start=(i == 0),
                                stop=(i == len(group) - 1),
                            )
                        fb_ap = fb_sb[:, b, l, oc:oc + 1]
                        dst = osb[:, oc, h0:h0 + HH, :]
                        last = sample_idx[0] == BS * L - 1
                        # last sample: final half-add lands on vector (its
                        # queue is free) so the tail isn't serialized
                        on_v = (half == 0) if not last else ((oc + half) != 1)
                        if on_v:
                            nc.vector.tensor_scalar_add(dst, ps[:], fb_ap)
                        else:
                            nc.scalar.add(dst, ps[:], fb_ap)
                        if last:
                            # tail: store each half as soon as it's added,
                            # all on the fast sync queue (halves arrive
                            # every ~3.7us > the ~2.7us transfer)
                            nc.sync.dma_start(
                                out=out_d[b, l, :, oc, h0:h0 + HH, :],
                                in_=osb[:, oc, h0:h0 + HH, :],
                            )
                        gidx += 1
                        if hooks and gidx in hooks:
                            hooks[gidx]()
                    if sample_idx[0] < BS * L - 1:
                        # gpsimd's software DMA is ~2x slower: it gets the
                        # even samples only, sync carries the rest
                        st_eng = (nc.gpsimd if sample_idx[0] in (0, 2, 4)
                                  else nc.sync)
                        st_eng.dma_start(
                            out=out_d[b, l, :, oc], in_=osb[:, oc]
                        )
                sample_idx[0] += 1

            conv_sample(0, 0)

            # b1 calib + scales (frames pooled during b0l0's conv)
            calib(1, 1, 0, L)
            for l in range(L):
                scale_x(1, l, 0, "v")
                scale_x(1, l, 1, "s")

            def tail_calib():
                # b0 l2/l3 calib + joint gate matmul part
                calib(0, 1, 2, 2)
                for l in (2, 3):
                    scale_x(0, l, 0, "v")
                    scale_x(0, l, 1, "s")

            def tail_gate():
                gate(0, BS, 0, L,
                     [(0, 2), (0, 3)] + [(1, l) for l in range(L)])

            conv_sample(0, 1, hooks={2: tail_calib, 3: tail_gate})
            conv_sample(0, 2)
            conv_sample(0, 3)
            for l in range(L):
                conv_sample(1, l)

    _split_excess_waits(nc)
    return nc


def kernel(x, weight, bias, tconv_w, tconv_b, fc_w, fc_b):
    global _last_results
    x = np.asarray(x, dtype=np.float32)
    weight = np.asarray(weight, dtype=np.float32)
    bias = np.asarray(bias, dtype=np.float32)
    tconv_w = np.asarray(tconv_w, dtype=np.float32)
    tconv_b = np.asarray(tconv_b, dtype=np.float32)
    fc_w = np.asarray(fc_w, dtype=np.float32)
    fc_b = np.asarray(fc_b, dtype=np.float32)

    HW = H * W
    # host-side packing (shared across cores); 1/(H*W) pooling norm and
    # the +1 biases folded here
    x_bf = x.astype(np_bf16).reshape(B, L, CC, 128, HW)
    w_host = np.ascontiguousarray(
        weight.transpose(1, 2, 3, 0)
        .reshape(CC, 128, 9, OC, 128)
        .transpose(1, 3, 0, 2, 4)
        .astype(np_bf16)
    )
    inv = np.float32(1.0 / HW)
    tcw = (tconv_w * inv).transpose(1, 2, 0)          # (CIN_in, 3, CIN_out)
    fcw = (fc_w[0] * inv)[:, :, None]                 # (CIN_in, 3, 1)
    tcw_host = np.ascontiguousarray(
        np.concatenate([tcw, fcw], axis=2)
        .reshape(CC, 128, 3, CIN + 1)
        .transpose(1, 0, 2, 3)
        .astype(np_bf16)
    )
    sm_host = np.ascontiguousarray(np.concatenate([
        tconv_b.reshape(CC, 128).T + np.float32(1.0),
        bias.reshape(OC, 128).T,
        np.full((128, 1), fc_b[0] + 1.0, dtype=np.float32),
    ], axis=1))

    nc = build_nc()
    in_maps = []
    for core in range(NCORES):
        xc = x_bf[core * BS:(core + 1) * BS]          # (BS, L, CC, 128, HW)
        in_maps.append({
            "x0": np.ascontiguousarray(xc[0].transpose(0, 2, 1, 3)),
            "x1": np.ascontiguousarray(xc[1].transpose(2, 0, 1, 3)),
            "w": w_host,
            "tcwfcw": tcw_host,
            "smalls": sm_host,
        })
    res = run_bass_kernel_spmd(nc, in_maps, core_ids=list(range(NCORES)))
    _last_results = res
    # out_d is [BS, L, 128, OC, H, W] partition-major -> un-permute
    outs = []
    for r in res.results:
        o = r["out"].reshape(BS, L, 128, OC, HW).transpose(0, 1, 3, 2, 4)
        outs.append(np.ascontiguousarray(o).reshape(BS * L, COUT, H, W))
    return np.concatenate(outs, axis=0)
